# revision 19
# baseline (speedup 1.0000x reference)
"""Single-head causal attention (B=16, S=2048, D=1024, H=128) on 8 TRN2 cores.

Strategy: batch-parallel across cores (2 batches per core), weights replicated.
Per core, per batch:
  stage 1: DMA X (fp32) -> convert bf16 (DVE) -> xbar-transpose to X^T laid
           out [p, c, s] with d = c*128 + p (weights reshaped to match).
  stage 2: Q^T,K^T,V^T = W^T X^T on PE (bf16 in, fp32 accum), evacuated to
           bf16 SBUF on the Scalar engine; V^T -> V via PE transpose.
  stage 3: flash-style attention in transposed layout:
           S^T[t,q] = K_j Q^T (PE) -> exp on ACT (psum->sbuf bf16, causal mask
           on diagonal blocks via DVE multiply) -> out^T[h,q] += V_j^T expT and
           den[1,q] += ones^T expT (PE) -> per-window epilogue: PE transpose of
           den and out^T back to natural layout, DVE reciprocal, normalization
           fused into PSUM eviction, DMA out.
"""

import numpy as np
import ml_dtypes

import concourse.bass as bass
import concourse.bacc as bacc
import concourse.mybir as mybir
from concourse import tile
from concourse.tile import add_dep_helper
from concourse.bass_utils import run_bass_kernel_spmd

F32 = mybir.dt.float32
BF16 = mybir.dt.bfloat16
PSUM = bass.MemorySpace.PSUM
Exp = mybir.ActivationFunctionType.Exp

P = 128          # partition dim / head size / tile unit
D = 1024         # model dim
H = 128          # head size
DW = D // P      # 8 d-groups
N_CORES = 8
N_WARMUP = 72    # kernel-start PE warmup matmuls


def build_nc(BSH, S, SW=512):
    """Build the per-core Bass program. BSH = batches per core."""
    NW = S // SW      # q windows
    NT = S // P       # 128-row tiles in S
    WPT = SW // P     # q tiles per window
    ISQ = float(1.0 / np.sqrt(H))

    nc = bacc.Bacc("TRN2", target_bir_lowering=False, debug=False)

    x_d = nc.dram_tensor("x", [BSH, S, D], F32, kind="ExternalInput")
    w_d = {
        name: nc.dram_tensor(name, [P, DW, H], BF16, kind="ExternalInput")
        for name in ("wq", "wk", "wv")
    }
    mask_d = nc.dram_tensor("mask", [P, P], BF16, kind="ExternalInput")
    id32_d = nc.dram_tensor("id32", [P, P], F32, kind="ExternalInput")
    id16_d = nc.dram_tensor("id16", [P, P], BF16, kind="ExternalInput")
    ones_d = nc.dram_tensor("ones", [P, 1], BF16, kind="ExternalInput")
    out_d = nc.dram_tensor("out", [BSH, S, H], F32, kind="ExternalOutput")

    with tile.TileContext(nc) as tc:
        from contextlib import ExitStack

        with ExitStack() as ctx:
            cpool = ctx.enter_context(tc.tile_pool(name="consts", bufs=1))
            w_sb = {}
            for name in ("wq", "wk", "wv"):
                t = cpool.tile([P, DW, H], BF16, tag=name)
                nc.scalar.dma_start(t[:], w_d[name].ap())
                w_sb[name] = t
            mask_sb = cpool.tile([P, P], BF16, tag="mask")
            nc.scalar.dma_start(mask_sb[:], mask_d.ap())
            id32_sb = cpool.tile([P, P], F32, tag="id32")
            nc.scalar.dma_start(id32_sb[:], id32_d.ap())
            id16_sb = cpool.tile([P, P], BF16, tag="id16")
            nc.scalar.dma_start(id16_sb[:], id16_d.ap())
            ones_sb = cpool.tile([P, 1], BF16, tag="ones")
            nc.scalar.dma_start(ones_sb[:], ones_d.ap())

            big = ctx.enter_context(tc.tile_pool(name="big", bufs=2))
            stage = ctx.enter_context(tc.tile_pool(name="stage", bufs=4))
            epool = ctx.enter_context(tc.tile_pool(name="exp", bufs=18))
            opool = ctx.enter_context(tc.tile_pool(name="osb", bufs=2))
            spool = ctx.enter_context(tc.tile_pool(name="small", bufs=2))

            XT, QT, KT, VT, V = {}, {}, {}, {}, {}
            for b in range(BSH):
                # X^T layout: XT[p, c, s] = X[s, c*P + p]
                XT[b] = big.tile([P, DW, S], BF16, tag="xt", name=f"xt{b}")
                QT[b] = big.tile([P, S], BF16, tag="qt", name=f"qt{b}")
                KT[b] = big.tile([P, S], BF16, tag="kt", name=f"kt{b}")
                VT[b] = big.tile([P, S], BF16, tag="vt", name=f"vt{b}")
                V[b] = big.tile([P, NT, P], BF16, tag="v", name=f"v{b}")

            def xt_rhs(b, c, w):
                # moving operand [d-part, s] for d-group c, q window w;
                # contiguous in s so the PE streams at 1 column/cycle.
                return XT[b][:, c, w * SW:(w + 1) * SW]

            # ---- PE warmup: HAM starts at K=4/8 (1.2 GHz); run dummy
            # matmuls during the stage-1 lead-in so projections start at
            # full clock. z16 needs no DMA (memset), so these run
            # immediately.
            z16 = cpool.tile([P, SW], BF16, tag="z16")
            nc.gpsimd.memset(z16[:], 0.0)
            with tc.tile_pool(name="warm", bufs=1, space=PSUM) as wp:
                wps = wp.tile([P, SW], F32, tag="w")
                for _ in range(N_WARMUP):
                    nc.tensor.matmul(wps[:], z16[:, 0:P], z16[:],
                                     start=True, stop=True)

            # ---- stage 1 + projections, interleaved in chunk groups ----
            # The xbar transpose<->copy mode switch serializes ALL DMA
            # traffic, so transposes are batched: one xbar call per group of
            # chunks. in_ [128, n*D] transposes to logical [n*D, 128] whose
            # 128-row blocks are enumerated i = k*DW + c -> uniform 3D out
            # AP. Projections consume the corresponding window group right
            # after each xbar, c-outer to amortize LDWEIGHTS. Batch 0 uses
            # small leading groups for a fast pipeline start.
            GROUPS = {0: [(0, 4), (4, 8), (8, 16)],
                      1: [(0, 8), (8, 16)]} if BSH == 2 else {
                      b: [(0, NT)] for b in range(BSH)}

            def stage_in(b, k0, k1):
                x16f = stage.tile([P, (k1 - k0) * D], BF16, tag="x16f",
                                  bufs=2, name=f"x16f{b}_{k0}")
                for kk, k in enumerate(range(k0, k1)):
                    xs = stage.tile([P, D], F32, tag="xs", bufs=6)
                    nc.sync.dma_start(xs[:], x_d.ap()[b, k * P:(k + 1) * P, :])
                    nc.vector.tensor_copy(x16f[:, kk * D:(kk + 1) * D], xs[:])
                return x16f

            def stage_xbar(b, k0, k1, x16f):
                # consecutive per-chunk transposes (one xbar-mode stretch);
                # c-major output keeps the projection rhs contiguous in s.
                for kk, k in enumerate(range(k0, k1)):
                    nc.sync.dma_start_transpose(
                        XT[b][:, :, k * P:(k + 1) * P],
                        x16f[:, kk * D:(kk + 1) * D],
                    )

            def proj_group(b, k0, k1):
                pjc = tc.tile_pool(name=f"pj{b}_{k0}", bufs=4, space=PSUM)
                vtc = tc.tile_pool(name=f"vt{b}_{k0}", bufs=2, space=PSUM)
                with pjc as pj, vtc as vtp:
                    _proj_group(b, k0, k1, pj, vtp)

            def _proj_group(b, k0, k1, pj, vtp):
                ws = range(k0 // WPT, k1 // WPT)
                for wname, dst in (("wq", QT[b]), ("wk", KT[b]),
                                   ("wv", VT[b])):
                    ps = {w: pj.tile([P, SW], F32, tag="ps", name=f"ps{w}")
                          for w in ws}
                    for c in range(DW):
                        for w in ws:
                            nc.tensor.matmul(
                                ps[w][:],
                                w_sb[wname][:, c, :],
                                xt_rhs(b, c, w),
                                start=(c == 0),
                                stop=(c == DW - 1),
                            )
                    for w in ws:
                        nc.scalar.copy(dst[:, w * SW:(w + 1) * SW], ps[w][:])
                for t in range(k0, k1):
                    vp = vtp.tile([P, P], BF16, tag="vp")
                    nc.tensor.transpose(
                        vp[:], VT[b][:, t * P:(t + 1) * P], id16_sb[:]
                    )
                    nc.scalar.copy(V[b][:, t, :], vp[:])

            # in/xbar phase schedule per batch: consecutive "in" phases
            # share one copy-mode stretch; each sublist of xbars shares one
            # transpose-mode stretch -> few mode switches.
            SCHED = {0: [("in", 0, 4), ("in", 4, 8), ("xb", 0, 4),
                         ("xb", 4, 8), ("in", 8, 16), ("xb", 8, 16)],
                     1: [("in", 0, 8), ("in", 8, 16), ("xb", 0, 8),
                         ("xb", 8, 16)]}

            def stage_batch(b):
                if BSH != 2:
                    x = stage_in(b, 0, NT)
                    stage_xbar(b, 0, NT, x)
                    return
                bufs = {}
                for op, k0, k1 in SCHED[b]:
                    if op == "in":
                        bufs[k0] = stage_in(b, k0, k1)
                    else:
                        stage_xbar(b, k0, k1, bufs[k0])

            def proj_batch(b):
                for (k0, k1) in GROUPS[b]:
                    proj_group(b, k0, k1)

            # ---- stage 3: attention ----
            def attn_windows(b, wlo, whi):
                with tc.tile_pool(name=f"sc{b}_{wlo}", bufs=2, space=PSUM) as scp, \
                     tc.tile_pool(name=f"ou{b}_{wlo}", bufs=2, space=PSUM) as oup, \
                     tc.tile_pool(name=f"de{b}_{wlo}", bufs=2, space=PSUM) as dep, \
                     tc.tile_pool(name=f"tr{b}_{wlo}", bufs=2, space=PSUM) as trp:
                    for w in range(wlo, whi):
                        nj = WPT * w + WPT
                        outp = oup.tile([P, SW], F32, tag="o")
                        denp = dep.tile([1, SW], F32, tag="d")
                        sp = {}
                        es = {}

                        def scores(j, b=b, w=w, sp=sp):
                            c0 = max(0, j - WPT * w)
                            s = scp.tile([P, SW], F32, tag="s")
                            nc.tensor.matmul(
                                s[:, c0 * P:],
                                KT[b][:, j * P:(j + 1) * P],
                                QT[b][:, w * SW + c0 * P:(w + 1) * SW],
                                start=True,
                                stop=True,
                            )
                            sp[j] = s

                        scores(0)
                        for j in range(nj):
                            c0 = max(0, j - WPT * w)
                            e = epool.tile([P, SW], BF16, tag="e")
                            if c0 > 0:
                                nc.gpsimd.memset(e[:, 0:c0 * P], 0.0)
                            nc.scalar.activation(
                                e[:, c0 * P:], sp[j][:, c0 * P:], Exp, scale=ISQ
                            )
                            if j >= WPT * w:
                                nc.gpsimd.tensor_mul(
                                    e[:, c0 * P:(c0 + 1) * P],
                                    e[:, c0 * P:(c0 + 1) * P],
                                    mask_sb[:],
                                )
                            if j + 1 < nj:
                                scores(j + 1)
                            nc.tensor.matmul(
                                outp[:], V[b][:, j, :], e[:],
                                start=(j == 0), stop=(j == nj - 1),
                            )
                            es[j] = e

                        # den sweep: single ones-LDWEIGHTS for the window
                        for j in range(nj):
                            nc.tensor.matmul(
                                denp[:], ones_sb[:], es[j][:],
                                start=(j == 0), stop=(j == nj - 1),
                            )

                        # window epilogue
                        dT = spool.tile([1, SW], F32, tag="dT")
                        nc.scalar.copy(dT[:], denp[:])
                        dtr = trp.tile([P, WPT], F32, tag="tr")
                        for c in range(WPT):
                            nc.tensor.transpose(
                                dtr[:, c:c + 1], dT[:, c * P:(c + 1) * P],
                                id32_sb[0:1, 0:1],
                            )
                        rec = spool.tile([P, WPT], F32, tag="rec")
                        nc.vector.reciprocal(rec[:], dtr[:])
                        oT = spool.tile([P, SW], F32, tag="oT")
                        nc.scalar.copy(oT[:], outp[:])
                        osb = opool.tile([P, WPT, P], F32, tag="osb")
                        for c in range(WPT):
                            otr = trp.tile([P, P], F32, tag="tr")
                            nc.tensor.transpose(
                                otr[:], oT[:, c * P:(c + 1) * P], id32_sb[:]
                            )
                            nc.vector.tensor_scalar_mul(
                                osb[:, c, :], otr[:], rec[:, c:c + 1]
                            )
                        nc.sync.dma_start(
                            out_d.ap()[b, w * SW:(w + 1) * SW, :].rearrange(
                                "(c r) h -> r c h", r=P
                            ),
                            osb[:],
                        )

            # ---- driver: interleave so PE never waits on staging ----
            if BSH == 2:
                stage_batch(0)                   # sync q: g0,g1,g2 xbars
                proj_group(0, 0, 4)              # windows 0
                proj_group(0, 4, 8)              # window 1
                stage_batch(1)                   # queues behind b0 staging
                attn_windows(0, 0, 2)            # runs while b0g2/b1 stage
                proj_group(0, 8, 16)             # windows 2,3
                attn_windows(0, 2, NW)
                proj_batch(1)
                attn_windows(1, 0, NW)
            else:
                for b in range(BSH):
                    stage_batch(b)
                for b in range(BSH):
                    proj_batch(b)
                    attn_windows(b, 0, NW)

    nc.compile()
    return nc


def make_consts():
    bf16 = ml_dtypes.bfloat16
    mask = np.triu(np.ones((P, P), dtype=np.float32)).astype(bf16)
    id32 = np.eye(P, dtype=np.float32)
    id16 = np.eye(P, dtype=np.float32).astype(bf16)
    ones = np.ones((P, 1), dtype=np.float32).astype(bf16)
    return mask, id32, id16, ones


def prep_weights(Wq, Wk, Wv):
    bf16 = ml_dtypes.bfloat16
    out = {}
    for name, W in (("wq", Wq), ("wk", Wk), ("wv", Wv)):
        # X^T xbar layout: XT[p, c, s] = X[s, c*P + p], so weight row d=c*P+p
        # must sit at [p, c, h].
        out[name] = np.ascontiguousarray(
            np.asarray(W, dtype=np.float32).reshape(DW, P, H).transpose(1, 0, 2)
        ).astype(bf16)
    return out


_NC_CACHE = {}


def _get_nc(BSH, S, SW=512):
    key = (BSH, S, SW)
    if key not in _NC_CACHE:
        _NC_CACHE[key] = build_nc(BSH, S, SW)
    return _NC_CACHE[key]


def kernel(input, Wq, Wk, Wv):
    input = np.asarray(input, dtype=np.float32)
    B, S, D_ = input.shape
    assert D_ == D and B % N_CORES == 0
    BSH = B // N_CORES

    nc = _get_nc(BSH, S)
    wmaps = prep_weights(Wq, Wk, Wv)
    mask, id32, id16, ones = make_consts()

    in_maps = []
    for i in range(N_CORES):
        m = {
            "x": np.ascontiguousarray(input[i * BSH:(i + 1) * BSH]),
            "mask": mask, "id32": id32, "id16": id16, "ones": ones,
        }
        m.update(wmaps)
        in_maps.append(m)

    res = run_bass_kernel_spmd(nc, in_maps, core_ids=list(range(N_CORES)))
    out = np.concatenate([res.results[i]["out"] for i in range(N_CORES)], axis=0)
    return out
